# revision 6
# baseline (speedup 1.0000x reference)
"""ColumnBlock (per-position-weight transformer block) on 8 trn2 NeuronCores.

Sharding: seq_len (N=1024) column axis split 8 ways (128 positions/core).
Each core: norm1 + q/k projections for its positions, AllGather of h (values/
keys input) and k across cores, full-row attention for its 128 query rows,
norm2 + per-position MLP. Final residual (x2 + mlp_out + b2) is assembled on
the host during unshard (mlp output is produced in c-major layout).
"""

import numpy as np

import concourse.bass as bass
import concourse.mybir as mybir
import concourse.tile as tile
from concourse import bacc
from concourse.bass_utils import run_bass_kernel_spmd
from concourse.masks import make_identity

B, N, C, D = 32, 1024, 384, 48
NC_ = 8           # cores
NL = N // NC_     # positions per core = 128
CH = C // 128     # c chunks = 3
EPS = 1e-5
SCALE = C ** -0.5
F32 = mybir.dt.float32
F32R = mybir.dt.float32r


def r(ap):
    return ap  # fp32 matmuls: fp32r requires producer-side rounding


def _norm(nc, sb, x_src_b, w_sb, b_sb, out_b, eps_sb):
    """LayerNorm over free axis for one b-slice. x_src_b/out_b: [128, C] SBUF.
    w_sb/b_sb: [128, C]. scr: scratch pool."""
    red = sb.tile([128, 1], F32, tag="nrm_red")
    mu = sb.tile([128, 1], F32, tag="nrm_mu")
    ss = sb.tile([128, 1], F32, tag="nrm_ss")
    rstd = sb.tile([128, 1], F32, tag="nrm_rstd")
    xc = sb.tile([128, C], F32, tag="nrm_xc")
    nc.vector.reduce_sum(red[:], x_src_b, axis=mybir.AxisListType.X)
    nc.vector.tensor_scalar_mul(mu[:], red[:], 1.0 / C)
    nc.vector.tensor_scalar_sub(xc[:], x_src_b, mu[:])
    nc.scalar.activation(out_b, xc[:], mybir.ActivationFunctionType.Square,
                         accum_out=ss[:])
    # rstd = 1/sqrt(ss/C + eps)
    nc.scalar.activation(rstd[:], ss[:], mybir.ActivationFunctionType.Sqrt,
                         bias=eps_sb, scale=1.0 / C)
    nc.vector.reciprocal(rstd[:], rstd[:])
    nc.vector.tensor_scalar_mul(xc[:], xc[:], rstd[:])
    nc.vector.tensor_mul(xc[:], xc[:], w_sb)
    nc.vector.tensor_add(out_b, xc[:], b_sb)


def build_kernel():
    nc = bacc.Bacc("TRN2", num_devices=NC_)

    # ---- per-core external inputs (host pre-sharded / pre-laid-out) ----
    x_l = nc.dram_tensor("x_l", [B, NL, C], F32, kind="ExternalInput")
    n1w = nc.dram_tensor("n1w", [NL, C], F32, kind="ExternalInput")
    n1b = nc.dram_tensor("n1b", [NL, C], F32, kind="ExternalInput")
    n2w = nc.dram_tensor("n2w", [NL, C], F32, kind="ExternalInput")
    n2b = nc.dram_tensor("n2b", [NL, C], F32, kind="ExternalInput")
    # [128c, ch, n, d] packed
    wq_t = nc.dram_tensor("wq_t", [128, CH * NL * D], F32, kind="ExternalInput")
    wk_t = nc.dram_tensor("wk_t", [128, CH * NL * D], F32, kind="ExternalInput")
    w1_t = nc.dram_tensor("w1_t", [128, CH * NL * D], F32, kind="ExternalInput")
    # [48d, ch, n, 128c] packed
    w2_t = nc.dram_tensor("w2_t", [D, CH * NL * 128], F32, kind="ExternalInput")
    bq_t = nc.dram_tensor("bq_t", [D, NL], F32, kind="ExternalInput")
    bk_t = nc.dram_tensor("bk_t", [D, NL], F32, kind="ExternalInput")
    b1_t = nc.dram_tensor("b1_t", [D, NL], F32, kind="ExternalInput")

    # ---- outputs ----
    attn_l = nc.dram_tensor("attn_l", [B, NL, N], F32, kind="ExternalOutput")
    x2_l = nc.dram_tensor("x2_l", [NL, B, C], F32, kind="ExternalOutput")
    mlp_l = nc.dram_tensor("mlp_l", [C, NL, B], F32, kind="ExternalOutput")

    with tile.TileContext(nc) as tc:
        dram = tc.alloc_tile_pool(name="dram", bufs=1, space="DRAM")
        sb = tc.alloc_tile_pool(name="sb", bufs=3)
        sb2 = tc.alloc_tile_pool(name="sb2", bufs=2)
        pers = tc.alloc_tile_pool(name="pers", bufs=1)
        psA = tc.alloc_tile_pool(name="psA", bufs=2, space="PSUM")
        psB = tc.alloc_tile_pool(name="psB", bufs=2, space="PSUM")
        psC = tc.alloc_tile_pool(name="psC", bufs=3, space="PSUM")

        h_bounce = dram.tile([1, NL, B, C], F32)
        h_full = dram.tile([NC_, NL, B, C], F32)
        k_bounce = dram.tile([1, D, B, NL], F32)
        k_full = dram.tile([NC_, D, B, NL], F32)
        x2_d = dram.tile([NL, B, C], F32)

        ident = pers.tile([128, 128], F32, tag="ident")
        make_identity(nc, ident[:])
        eps_sb = pers.tile([128, 1], F32, tag="eps")
        nc.gpsimd.memset(eps_sb[:], EPS)

        # persistent SBUF
        h_T = pers.tile([128, CH, NL, B], F32, tag="h_T")      # 49KB/part
        q_T = pers.tile([D, B, NL], F32, tag="q_T")            # 16KB
        m_T = pers.tile([D, B, NL], F32, tag="m_T")            # 16KB

        n1w_sb = pers.tile([128, C], F32, tag="n1w")
        n1b_sb = pers.tile([128, C], F32, tag="n1b")
        n2w_sb = pers.tile([128, C], F32, tag="n2w")
        n2b_sb = pers.tile([128, C], F32, tag="n2b")
        bq_sb = pers.tile([D, NL], F32, tag="bq")
        bk_sb = pers.tile([D, NL], F32, tag="bk")
        b1_sb = pers.tile([D, NL], F32, tag="b1")
        nc.sync.dma_start(n1w_sb[:], n1w[:])
        nc.sync.dma_start(n1b_sb[:], n1b[:])
        nc.sync.dma_start(n2w_sb[:], n2w[:])
        nc.sync.dma_start(n2b_sb[:], n2b[:])
        nc.sync.dma_start(bq_sb[:], bq_t[:])
        nc.sync.dma_start(bk_sb[:], bk_t[:])
        nc.sync.dma_start(b1_sb[:], b1_t[:])

        # ---- stage A: norm1 per b; h -> bounce + h_T via PE transpose ----
        for b in range(B):
            x_b = sb.tile([128, C], F32, tag="x_b")
            nc.sync.dma_start(x_b[:], x_l[:, :, :].rearrange("b n c -> n b c")[:, b, :])
            h_b = sb.tile([128, C], F32, tag="h_b")
            _norm(nc, sb, x_b[:], n1w_sb[:], n1b_sb[:], h_b[:], eps_sb[:])
            nc.sync.dma_start(h_bounce[0, :, b, :], h_b[:])
            for ch in range(CH):
                tp = psB.tile([128, 128], F32, tag="tp")
                nc.tensor.transpose(tp[:], h_b[:, ch * 128:(ch + 1) * 128], ident[:])
                nc.vector.tensor_copy(h_T[:, ch, :, b], tp[:])

        nc.gpsimd.collective_compute(
            "AllGather", mybir.AluOpType.bypass,
            replica_groups=[list(range(NC_))],
            ins=[h_bounce[:].opt()], outs=[h_full[:].opt()],
        )

        # ---- stage C: q/k projections (per position) ----
        k_T = m_T
        for which, wdram, out_t, bias_sb, scl in (
            ("q", wq_t, q_T, bq_sb, SCALE), ("k", wk_t, k_T, bk_sb, 1.0),
        ):
            for g in range(4):   # 32 positions per weight chunk
                wg = sb2.tile([128, CH, 32, D], F32, tag="wt")
                nc.sync.dma_start(
                    wg[:], wdram[:].rearrange("p (ch n d) -> p ch n d", ch=CH, n=NL)
                    [:, :, g * 32:(g + 1) * 32, :])
                for j2 in range(2):  # 16 n per psum bank
                    ps = psC.tile([128, 512], F32, tag="pc")
                    psv = ps[:D, :].rearrange("d (n b) -> d n b", n=16)
                    for j in range(16):
                        n_ = g * 32 + j2 * 16 + j
                        for ch in range(CH):
                            nc.tensor.matmul(
                                psv[:, j, :], r(wg[:, ch, j2 * 16 + j, :]),
                                r(h_T[:, ch, n_, :]),
                                start=(ch == 0), stop=(ch == CH - 1))
                    nsl = slice(g * 32 + j2 * 16, g * 32 + j2 * 16 + 16)
                    nc.vector.tensor_scalar_mul(
                        out_t[:, :, nsl], psv.rearrange("d n b -> d b n"), scl)
            nc.vector.tensor_add(
                out_t[:], out_t[:],
                bias_sb[:, None, :].to_broadcast([D, B, NL]))

        nc.sync.dma_start(k_bounce[0], k_T[:])
        nc.gpsimd.collective_compute(
            "AllGather", mybir.AluOpType.bypass,
            replica_groups=[list(range(NC_))],
            ins=[k_bounce[:].opt()], outs=[k_full[:].opt()],
        )

        # ---- stage E: attention per b ----
        for b in range(B):
            k_b = sb2.tile([D, NC_, NL], F32, tag="k_b")
            nc.gpsimd.dma_start(k_b[:], k_full[:, :, b, :].rearrange("s d m -> d s m"))

            e_q = sb2.tile([128, N], F32, tag="e_q")
            for half in range(2):
                sq_ps = psA.tile([128, 512], F32, tag="sq")
                nc.tensor.matmul(
                    sq_ps[:], r(q_T[:, b, :]),
                    r(k_b[:, half * 4:(half + 1) * 4, :]),
                    start=True, stop=True)
                nc.scalar.activation(e_q[:, half * 512:(half + 1) * 512], sq_ps[:],
                                     mybir.ActivationFunctionType.Exp)
            den = sb.tile([128, 1], F32, tag="den")
            rden = sb.tile([128, 1], F32, tag="rden")
            nc.vector.reduce_sum(den[:], e_q[:], axis=mybir.AxisListType.X)
            nc.vector.reciprocal(rden[:], den[:])
            attn_sb = sb2.tile([128, N], F32, tag="attn_sb")
            nc.scalar.activation(attn_sb[:], e_q[:],
                                 mybir.ActivationFunctionType.Copy, scale=rden[:])
            nc.sync.dma_start(attn_l[b], attn_sb[:])

            e_T = sb2.tile([128, NC_, 128], F32, tag="e_T")
            for ch in range(NC_):
                st_ps = psB.tile([128, 128], F32, tag="tp")
                nc.tensor.matmul(st_ps[:], r(k_b[:, ch, :]), r(q_T[:, b, :]),
                                 start=True, stop=True)
                nc.scalar.activation(e_T[:, ch, :], st_ps[:],
                                     mybir.ActivationFunctionType.Exp)
            av_ps = psC.tile([128, 512], F32, tag="pc")
            for ch in range(NC_):
                h_v = sb2.tile([128, C], F32, tag="h_v")
                nc.gpsimd.dma_start(h_v[:], h_full[ch, :, b, :])
                nc.tensor.matmul(av_ps[:, :C], r(e_T[:, ch, :]), r(h_v[:]),
                                 start=(ch == 0), stop=(ch == NC_ - 1))
            x2_b = sb.tile([128, C], F32, tag="x2_b")
            nc.scalar.activation(x2_b[:], av_ps[:, :C],
                                 mybir.ActivationFunctionType.Copy, scale=rden[:])
            xr_b = sb.tile([128, C], F32, tag="x_b")
            nc.sync.dma_start(xr_b[:], x_l[:, :, :].rearrange("b n c -> n b c")[:, b, :])
            nc.vector.tensor_add(x2_b[:], x2_b[:], xr_b[:])
            nc.sync.dma_start(x2_l[:, b, :], x2_b[:])
            nc.sync.dma_start(x2_d[:, b, :], x2_b[:])

        # ---- stage F: norm2 + transpose h2 into h_T (overwrite) ----
        for b in range(B):
            x2r = sb.tile([128, C], F32, tag="x2r")
            nc.sync.dma_start(x2r[:], x2_d[:, b, :])
            h2_b = sb.tile([128, C], F32, tag="h_b")
            _norm(nc, sb, x2r[:], n2w_sb[:], n2b_sb[:], h2_b[:], eps_sb[:])
            for ch in range(CH):
                tp = psB.tile([128, 128], F32, tag="tp")
                nc.tensor.transpose(tp[:], h2_b[:, ch * 128:(ch + 1) * 128], ident[:])
                nc.vector.tensor_copy(h_T[:, ch, :, b], tp[:])

        # ---- stage G: mlp fc1 + gelu ----
        for g in range(4):
            wg = sb2.tile([128, CH, 32, D], F32, tag="wt")
            nc.sync.dma_start(
                wg[:], w1_t[:].rearrange("p (ch n d) -> p ch n d", ch=CH, n=NL)
                [:, :, g * 32:(g + 1) * 32, :])
            for j2 in range(2):
                ps = psC.tile([128, 512], F32, tag="pc")
                psv = ps[:D, :].rearrange("d (n b) -> d n b", n=16)
                for j in range(16):
                    n_ = g * 32 + j2 * 16 + j
                    for ch in range(CH):
                        nc.tensor.matmul(
                            psv[:, j, :], r(wg[:, ch, j2 * 16 + j, :]),
                            r(h_T[:, ch, n_, :]),
                            start=(ch == 0), stop=(ch == CH - 1))
                nsl = slice(g * 32 + j2 * 16, g * 32 + j2 * 16 + 16)
                nc.vector.tensor_copy(m_T[:, :, nsl], psv.rearrange("d n b -> d b n"))
        nc.vector.tensor_add(m_T[:], m_T[:],
                             b1_sb[:, None, :].to_broadcast([D, B, NL]))
        nc.scalar.activation(m_T[:], m_T[:], mybir.ActivationFunctionType.Gelu)

        # ---- stage H: mlp fc2 (c-major out; residual+b2 on host) ----
        for g in range(16):  # 8 n per group
            w2g = sb2.tile([D, CH, 8, 128], F32, tag="wt")
            nc.sync.dma_start(
                w2g[:], w2_t[:].rearrange("d (ch n c) -> d ch n c", ch=CH, n=NL)
                [:, :, g * 8:(g + 1) * 8, :])
            for ch in range(CH):
                ps = psC.tile([128, 512], F32, tag="pc")
                psv = ps[:, :256].rearrange("c (n b) -> c n b", n=8)
                for j in range(8):
                    nc.tensor.matmul(psv[:, j, :], r(w2g[:, ch, j, :]),
                                     r(m_T[:, :, g * 8 + j]),
                                     start=True, stop=True)
                o_sb = sb.tile([128, 8, B], F32, tag="o_sb")
                nc.vector.tensor_copy(o_sb[:], psv)
                nc.sync.dma_start(
                    mlp_l[ch * 128:(ch + 1) * 128, g * 8:(g + 1) * 8, :], o_sb[:])

        for p in (psC, psB, psA, pers, sb2, sb, dram):
            p.release()

    nc.compile()
    return nc


_NC_CACHE = None


def kernel(x, n1_w, n1_b, wq, bq, wk, bk, n2_w, n2_b, w1, b1, w2, b2):
    global _NC_CACHE
    if _NC_CACHE is None:
        _NC_CACHE = build_kernel()
    nc = _NC_CACHE

    x = np.asarray(x, np.float32)
    in_maps = []
    for rr in range(NC_):
        sl = slice(rr * NL, (rr + 1) * NL)
        wq_p = np.ascontiguousarray(
            np.asarray(wq[sl], np.float32).transpose(2, 0, 1)  # [C, n, d]
            .reshape(CH, 128, NL, D).transpose(1, 0, 2, 3).reshape(128, CH * NL * D))
        wk_p = np.ascontiguousarray(
            np.asarray(wk[sl], np.float32).transpose(2, 0, 1)
            .reshape(CH, 128, NL, D).transpose(1, 0, 2, 3).reshape(128, CH * NL * D))
        w1_p = np.ascontiguousarray(
            np.asarray(w1[sl], np.float32).transpose(2, 0, 1)
            .reshape(CH, 128, NL, D).transpose(1, 0, 2, 3).reshape(128, CH * NL * D))
        # w2[sl]: [n, C, D] -> [d, ch, n, c]
        w2_p = np.ascontiguousarray(
            np.asarray(w2[sl], np.float32).transpose(2, 0, 1)  # [D, n, C]
            .reshape(D, NL, CH, 128).transpose(0, 2, 1, 3).reshape(D, CH * NL * 128))
        in_maps.append({
            "x_l": np.ascontiguousarray(x[:, sl, :]),
            "n1w": np.ascontiguousarray(np.asarray(n1_w[sl], np.float32)),
            "n1b": np.ascontiguousarray(np.asarray(n1_b[sl], np.float32)),
            "n2w": np.ascontiguousarray(np.asarray(n2_w[sl], np.float32)),
            "n2b": np.ascontiguousarray(np.asarray(n2_b[sl], np.float32)),
            "wq_t": wq_p, "wk_t": wk_p, "w1_t": w1_p, "w2_t": w2_p,
            "bq_t": np.ascontiguousarray(np.asarray(bq[sl], np.float32).T),
            "bk_t": np.ascontiguousarray(np.asarray(bk[sl], np.float32).T),
            "b1_t": np.ascontiguousarray(np.asarray(b1[sl], np.float32).T),
        })

    res = run_bass_kernel_spmd(nc, in_maps, core_ids=list(range(NC_)))

    x_out = np.empty((B, N, C), np.float32)
    attn = np.empty((B, N, N), np.float32)
    for rr in range(NC_):
        sl = slice(rr * NL, (rr + 1) * NL)
        o = res.results[rr]
        attn[:, sl, :] = o["attn_l"]
        x2 = o["x2_l"].transpose(1, 0, 2)           # [B, NL, C]
        mlp = o["mlp_l"].transpose(2, 1, 0)         # [B, NL, C]
        x_out[:, sl, :] = x2 + mlp + np.asarray(b2[sl], np.float32)[None]
    return x_out, attn
